# revision 15
# baseline (speedup 1.0000x reference)
"""Channel-wise FC kernel for Trainium2 (8 NeuronCores, SPMD).

Problem: out[b,c] = x[b,c] @ weights[c].T + bias[c]
  x: (8, 32, 1024, 512) f32, weights: (32, 512, 512) f32, bias: (32, 512) f32

Sharding: channel-parallel — core i owns channels [4i, 4i+4). For each channel
the device computes YT[f, bn] = sum_g WT[g,f] * XT[g, bn] (+bias), i.e. the
output is produced f-major; the host does all layout transposes (free wrt HW
time). Device-side DRAM layouts are chosen so every DMA reads/writes long
contiguous per-partition lines:

  xt  [C_LOC, N_CHUNKS, P, GT*NCH]  xt[c,nb,p,gt*NCH+n] = x^T[c, gt*128+p, nb*NCH+n]
  wt  [C_LOC, P, GT*F]              wt[c,p,gt*F+f]      = W[c, f, gt*128+p]
  bias[P, C_LOC*FT]                 bias[p, c*FT+ft]    = bias[c, ft*128+p]
  out [C_LOC, N_CHUNKS, P, FT*NCH]  out[c,nb,p,ft*NCH+n] = Y^T[c, ft*128+p, nb*NCH+n]

Compute dtype bf16 (TensorE full rate), fp32 PSUM accumulation, bf16 stores
upcast on host.
"""

import os
import sys

for _p in ("/root/.axon_site/_ro/trn_rl_repo", "/opt/trn_rl_repo"):
    if os.path.isdir(_p) and _p not in sys.path:
        sys.path.append(_p)

import numpy as np
import ml_dtypes

B, C, N, F, G = 8, 32, 1024, 512, 512
NCORES = 8
C_LOC = C // NCORES          # 4 channels per core
BN = B * N                   # 8192 rows per channel
P = 128
GT = G // P                  # 4 contraction tiles
FT = F // P                  # 4 output-partition tiles
NCH = 2048                   # rows per x DMA chunk (16KB partition lines)
N_CHUNKS = BN // NCH         # 4
NSL = NCH // 512             # 512-row matmul slices per chunk

_BF16 = ml_dtypes.bfloat16

_compiled = None


def _build():
    import concourse.bacc as bacc
    import concourse.mybir as mybir
    import concourse.tile as tile

    BF16 = mybir.dt.bfloat16
    F32 = mybir.dt.float32

    nc = bacc.Bacc("TRN2", target_bir_lowering=False, debug=False)
    xt = nc.dram_tensor("xt", [C_LOC, N_CHUNKS, P, GT * NCH], BF16,
                        kind="ExternalInput")
    wt = nc.dram_tensor("wt", [C_LOC, P, GT * F], BF16, kind="ExternalInput")
    bias = nc.dram_tensor("bias", [P, C_LOC * FT], F32, kind="ExternalInput")
    out = nc.dram_tensor("out", [C_LOC, N_CHUNKS, P, FT * NCH], BF16,
                         kind="ExternalOutput")

    xt_ap = xt.ap()
    wt_ap = wt.ap()
    out_ap = out.ap()

    with tile.TileContext(nc) as tc:
        with (
            tc.tile_pool(name="wpool", bufs=2) as wpool,
            tc.tile_pool(name="xpool", bufs=4) as xpool,
            tc.tile_pool(name="opool", bufs=3) as opool,
            tc.tile_pool(name="bpool", bufs=1) as bpool,
            tc.tile_pool(name="psum", bufs=8, space="PSUM") as pspool,
        ):
            # --- PE warmup burst: ~10 dummy matmuls on scratch data run while
            # the first real DMAs are in flight, flipping the HAM clock gate
            # to 8/8 before real matmuls start.
            warm_sb = bpool.tile([P, 512], BF16)
            nc.vector.memset(warm_sb[:], 0.0)
            warm_ps = pspool.tile([P, 512], F32, tag="ps")
            for _ in range(14):
                nc.tensor.matmul(warm_ps[:], warm_sb[:, :P], warm_sb[:],
                                 start=True, stop=True)

            def evict(c, ft, src, dst):
                bcol = b_sb[:, c * FT + ft:c * FT + ft + 1]
                if ft % 2 == 0:
                    nc.scalar.activation(
                        dst, src, mybir.ActivationFunctionType.Identity,
                        bias=bcol,
                    )
                else:
                    nc.vector.tensor_scalar_add(dst, src, bcol)

            def mm_group(c, nb, ns, ft, w_sb, x_sb, o_sb):
                ps = pspool.tile([P, 512], F32, tag="ps",
                                 name=f"ps_{c}_{nb}_{ns}_{ft}")
                for gt in range(GT):
                    nc.tensor.matmul(
                        ps[:],
                        w_sb[:, gt * F + ft * P:gt * F + (ft + 1) * P],
                        x_sb[:, gt * NCH + ns * 512:gt * NCH + (ns + 1) * 512],
                        start=(gt == 0),
                        stop=(gt == GT - 1),
                    )
                evict(c, ft, ps[:],
                      o_sb[:, ft * NCH + ns * 512:ft * NCH + (ns + 1) * 512])

            b_sb = bpool.tile([P, C_LOC * FT], F32)
            for c in range(C_LOC):
                w_sb = wpool.tile([P, GT * F], BF16, tag="w", name=f"w_{c}")
                if c == 0:
                    # critical pieces first: the very first matmul only needs
                    # w[gt0] and x[gt0]
                    nc.sync.dma_start(w_sb[:, 0:F], wt_ap[c][:, 0:F])
                else:
                    nc.sync.dma_start(w_sb[:], wt_ap[c])
                for nb in range(N_CHUNKS):
                    x_sb = xpool.tile([P, GT * NCH], BF16, tag="x",
                                      name=f"x_{c}_{nb}")
                    if c == 0 and nb == 0:
                        # (w_gt, x_gt) pairs land in consumption order; bias last
                        for gt in range(GT):
                            if gt > 0:
                                nc.sync.dma_start(
                                    w_sb[:, gt * F:(gt + 1) * F],
                                    wt_ap[c][:, gt * F:(gt + 1) * F],
                                )
                            nc.sync.dma_start(
                                x_sb[:, gt * NCH:(gt + 1) * NCH],
                                xt_ap[c, nb][:, gt * NCH:(gt + 1) * NCH],
                            )
                        nc.sync.dma_start(b_sb[:], bias.ap())
                    else:
                        nc.sync.dma_start(x_sb[:], xt_ap[c, nb])
                    o_sb = opool.tile([P, FT * NCH], BF16, tag="o",
                                      name=f"o_{c}_{nb}")
                    last = (c == C_LOC - 1 and nb == N_CHUNKS - 1)
                    if not last:
                        for ns in range(NSL):
                            for ft in range(FT):
                                mm_group(c, nb, ns, ft, w_sb, x_sb, o_sb)
                        # store via the GpSimd SWDGE ring so stores never
                        # head-of-line block x loads on the sync HWDGE FIFO
                        # (final-chunk stores stay on sync so no SWDGE work
                        # is left draining at the kernel tail)
                        nc.gpsimd.dma_start(out_ap[c, nb], o_sb[:])
                    else:
                        # ft-outer so each ft's store fires as soon as its
                        # slices are done -> short tail
                        for ft in range(FT):
                            for ns in range(NSL):
                                mm_group(c, nb, ns, ft, w_sb, x_sb, o_sb)
                                lo = ft * NCH + ns * 512
                                nc.sync.dma_start(
                                    out_ap[c, nb][:, lo:lo + 512],
                                    o_sb[:, lo:lo + 512],
                                )
    nc.compile()
    return nc


def _get_compiled():
    global _compiled
    if _compiled is None:
        _compiled = _build()
    return _compiled


def _shard_inputs(x, weights, bias):
    """Host-side: slice channels per core, bf16-cast, and pre-transpose into
    the device DRAM layouts documented at the top of this file."""
    x = np.asarray(x, dtype=np.float32)
    weights = np.asarray(weights, dtype=np.float32)
    bias = np.asarray(bias, dtype=np.float32)

    # (B, C, N, G) -> (C, G, B*N) -> (C, GT, P, N_CHUNKS, NCH) -> (C, nb, p, gt, n)
    xt_all = (
        x.transpose(1, 3, 0, 2)
        .reshape(C, GT, P, N_CHUNKS, NCH)
        .transpose(0, 3, 2, 1, 4)
        .reshape(C, N_CHUNKS, P, GT * NCH)
        .astype(_BF16)
    )
    # (C, F, G) -> W^T (C, G, F) -> (C, GT, P, F) -> (C, p, gt, F)
    wt_all = (
        weights.transpose(0, 2, 1)
        .reshape(C, GT, P, F)
        .transpose(0, 2, 1, 3)
        .reshape(C, P, GT * F)
        .astype(_BF16)
    )
    # (C, F) -> (C, FT, P) -> (P, C, FT)
    bias_all = (
        bias.reshape(C, FT, P).transpose(2, 0, 1).reshape(P, C * FT)
        .astype(np.float32)
    )

    in_maps = []
    for i in range(NCORES):
        sl = slice(i * C_LOC, (i + 1) * C_LOC)
        in_maps.append({
            "xt": np.ascontiguousarray(xt_all[sl]),
            "wt": np.ascontiguousarray(wt_all[sl]),
            "bias": np.ascontiguousarray(
                bias_all[:, i * C_LOC * FT:(i + 1) * C_LOC * FT]
            ),
        })
    return in_maps


def _unshard_output(results):
    # per-core out: (C_LOC, N_CHUNKS, P, FT*NCH) bf16
    yt = np.stack([np.asarray(r["out"]) for r in results])
    # (NCORES, C_LOC, nb, p, ft, n) -> (C, ft, p, nb, n) == (C, F, BN)
    yt = (
        yt.reshape(C, N_CHUNKS, P, FT, NCH)
        .transpose(0, 3, 2, 1, 4)
        .reshape(C, F, B, N)
    )
    y = yt.transpose(2, 0, 3, 1).astype(np.float32)  # (B, C, N, F)
    return np.ascontiguousarray(y)


def _ensure_axon_hooks():
    """bass_utils hard-imports antenv.axon_hooks when tracing is requested;
    some images lack that module. Shim it (with the ctypes NTFF hook when
    available) only if the real module is absent."""
    try:
        import antenv.axon_hooks  # noqa: F401
        return
    except ImportError:
        pass
    import types

    import antenv

    mod = types.ModuleType("antenv.axon_hooks")
    _hook = [None]
    mod.set_axon_ntff_profile_hook = lambda h: _hook.__setitem__(0, h)
    mod.get_axon_ntff_profile_hook = lambda: _hook[0]
    sys.modules["antenv.axon_hooks"] = mod
    antenv.axon_hooks = mod
    try:
        from trn_agent_boot.trn_boot import _ntff_profile_via_ctypes

        mod.set_axon_ntff_profile_hook(
            _ntff_profile_via_ctypes("/opt/axon/libaxon_pjrt.so")
        )
    except Exception:
        pass


def run_on_device(in_maps, **kwargs):
    _ensure_axon_hooks()
    from concourse.bass_utils import run_bass_kernel_spmd

    nc = _get_compiled()
    return run_bass_kernel_spmd(nc, in_maps, core_ids=list(range(NCORES)), **kwargs)


def kernel(x, weights, bias):
    in_maps = _shard_inputs(x, weights, bias)
    res = run_on_device(in_maps)
    return _unshard_output(res.results)


# revision 16
# speedup vs baseline: 1.0051x; 1.0051x over previous
"""Channel-wise FC kernel for Trainium2 (8 NeuronCores, SPMD).

Problem: out[b,c] = x[b,c] @ weights[c].T + bias[c]
  x: (8, 32, 1024, 512) f32, weights: (32, 512, 512) f32, bias: (32, 512) f32

Sharding: channel-parallel — core i owns channels [4i, 4i+4). For each channel
the device computes YT[f, bn] = sum_g WT[g,f] * XT[g, bn] (+bias), i.e. the
output is produced f-major; the host does all layout transposes (free wrt HW
time). Device-side DRAM layouts are chosen so every DMA reads/writes long
contiguous per-partition lines:

  xt  [C_LOC, N_CHUNKS, P, GT*NCH]  xt[c,nb,p,gt*NCH+n] = x^T[c, gt*128+p, nb*NCH+n]
  wt  [C_LOC, P, GT*F]              wt[c,p,gt*F+f]      = W[c, f, gt*128+p]
  bias[P, C_LOC*FT]                 bias[p, c*FT+ft]    = bias[c, ft*128+p]
  out [C_LOC, N_CHUNKS, P, FT*NCH]  out[c,nb,p,ft*NCH+n] = Y^T[c, ft*128+p, nb*NCH+n]

Compute dtype bf16 (TensorE full rate), fp32 PSUM accumulation, bf16 stores
upcast on host.
"""

import os
import sys

for _p in ("/root/.axon_site/_ro/trn_rl_repo", "/opt/trn_rl_repo"):
    if os.path.isdir(_p) and _p not in sys.path:
        sys.path.append(_p)

import numpy as np
import ml_dtypes

B, C, N, F, G = 8, 32, 1024, 512, 512
NCORES = 8
C_LOC = C // NCORES          # 4 channels per core
BN = B * N                   # 8192 rows per channel
P = 128
GT = G // P                  # 4 contraction tiles
FT = F // P                  # 4 output-partition tiles
NCH = 2048                   # rows per x DMA chunk (16KB partition lines)
N_CHUNKS = BN // NCH         # 4
NSL = NCH // 512             # 512-row matmul slices per chunk

_BF16 = ml_dtypes.bfloat16

_compiled = None


def _build():
    import concourse.bacc as bacc
    import concourse.mybir as mybir
    import concourse.tile as tile

    BF16 = mybir.dt.bfloat16
    F32 = mybir.dt.float32

    nc = bacc.Bacc("TRN2", target_bir_lowering=False, debug=False)
    xt = nc.dram_tensor("xt", [C_LOC, N_CHUNKS, P, GT * NCH], BF16,
                        kind="ExternalInput")
    wt = nc.dram_tensor("wt", [C_LOC, P, GT * F], BF16, kind="ExternalInput")
    bias = nc.dram_tensor("bias", [P, C_LOC * FT], F32, kind="ExternalInput")
    out = nc.dram_tensor("out", [C_LOC, N_CHUNKS, P, FT * NCH], BF16,
                         kind="ExternalOutput")

    xt_ap = xt.ap()
    wt_ap = wt.ap()
    out_ap = out.ap()

    with tile.TileContext(nc) as tc:
        with (
            tc.tile_pool(name="wpool", bufs=2) as wpool,
            tc.tile_pool(name="xpool", bufs=4) as xpool,
            tc.tile_pool(name="opool", bufs=3) as opool,
            tc.tile_pool(name="bpool", bufs=1) as bpool,
            tc.tile_pool(name="psum", bufs=8, space="PSUM") as pspool,
        ):
            # --- PE warmup burst: ~10 dummy matmuls on scratch data run while
            # the first real DMAs are in flight, flipping the HAM clock gate
            # to 8/8 before real matmuls start.
            warm_sb = bpool.tile([P, 512], BF16)
            nc.vector.memset(warm_sb[:], 0.0)
            warm_ps = pspool.tile([P, 512], F32, tag="ps")
            for _ in range(14):
                nc.tensor.matmul(warm_ps[:], warm_sb[:, :P], warm_sb[:],
                                 start=True, stop=True)

            def evict(c, ft, src, dst):
                bcol = b_sb[:, c * FT + ft:c * FT + ft + 1]
                if ft % 2 == 0:
                    nc.scalar.activation(
                        dst, src, mybir.ActivationFunctionType.Identity,
                        bias=bcol,
                    )
                else:
                    nc.vector.tensor_scalar_add(dst, src, bcol)

            def mm_group(c, nb, ns, ft, w_sb, x_sb, o_sb):
                ps = pspool.tile([P, 512], F32, tag="ps",
                                 name=f"ps_{c}_{nb}_{ns}_{ft}")
                for gt in range(GT):
                    nc.tensor.matmul(
                        ps[:],
                        w_sb[:, gt * F + ft * P:gt * F + (ft + 1) * P],
                        x_sb[:, gt * NCH + ns * 512:gt * NCH + (ns + 1) * 512],
                        start=(gt == 0),
                        stop=(gt == GT - 1),
                    )
                evict(c, ft, ps[:],
                      o_sb[:, ft * NCH + ns * 512:ft * NCH + (ns + 1) * 512])

            b_sb = bpool.tile([P, C_LOC * FT], F32)
            for c in range(C_LOC):
                w_sb = wpool.tile([P, GT * F], BF16, tag="w", name=f"w_{c}")
                if c == 0:
                    # critical pieces first: the very first matmul only needs
                    # w[gt0] and x[gt0]
                    nc.sync.dma_start(w_sb[:, 0:F], wt_ap[c][:, 0:F])
                else:
                    nc.sync.dma_start(w_sb[:], wt_ap[c])
                for nb in range(N_CHUNKS):
                    x_sb = xpool.tile([P, GT * NCH], BF16, tag="x",
                                      name=f"x_{c}_{nb}")
                    if c == 0 and nb == 0:
                        # (w_gt, x_gt) pairs land in consumption order; bias last
                        for gt in range(GT):
                            if gt > 0:
                                nc.sync.dma_start(
                                    w_sb[:, gt * F:(gt + 1) * F],
                                    wt_ap[c][:, gt * F:(gt + 1) * F],
                                )
                            nc.sync.dma_start(
                                x_sb[:, gt * NCH:(gt + 1) * NCH],
                                xt_ap[c, nb][:, gt * NCH:(gt + 1) * NCH],
                            )
                        nc.sync.dma_start(b_sb[:], bias.ap())
                    else:
                        nc.sync.dma_start(x_sb[:], xt_ap[c, nb])
                    o_sb = opool.tile([P, FT * NCH], BF16, tag="o",
                                      name=f"o_{c}_{nb}")
                    last = (c == C_LOC - 1 and nb == N_CHUNKS - 1)
                    if not last:
                        for ns in range(NSL):
                            for ft in range(FT):
                                mm_group(c, nb, ns, ft, w_sb, x_sb, o_sb)
                        nc.sync.dma_start(out_ap[c, nb], o_sb[:])
                    else:
                        # ft-outer so each ft's store fires as soon as its
                        # slices are done -> short tail
                        for ft in range(FT):
                            for ns in range(NSL):
                                mm_group(c, nb, ns, ft, w_sb, x_sb, o_sb)
                                lo = ft * NCH + ns * 512
                                nc.sync.dma_start(
                                    out_ap[c, nb][:, lo:lo + 512],
                                    o_sb[:, lo:lo + 512],
                                )
    nc.compile()
    return nc


def _get_compiled():
    global _compiled
    if _compiled is None:
        _compiled = _build()
    return _compiled


def _shard_inputs(x, weights, bias):
    """Host-side: slice channels per core, bf16-cast, and pre-transpose into
    the device DRAM layouts documented at the top of this file."""
    x = np.asarray(x, dtype=np.float32)
    weights = np.asarray(weights, dtype=np.float32)
    bias = np.asarray(bias, dtype=np.float32)

    # (B, C, N, G) -> (C, G, B*N) -> (C, GT, P, N_CHUNKS, NCH) -> (C, nb, p, gt, n)
    xt_all = (
        x.transpose(1, 3, 0, 2)
        .reshape(C, GT, P, N_CHUNKS, NCH)
        .transpose(0, 3, 2, 1, 4)
        .reshape(C, N_CHUNKS, P, GT * NCH)
        .astype(_BF16)
    )
    # (C, F, G) -> W^T (C, G, F) -> (C, GT, P, F) -> (C, p, gt, F)
    wt_all = (
        weights.transpose(0, 2, 1)
        .reshape(C, GT, P, F)
        .transpose(0, 2, 1, 3)
        .reshape(C, P, GT * F)
        .astype(_BF16)
    )
    # (C, F) -> (C, FT, P) -> (P, C, FT)
    bias_all = (
        bias.reshape(C, FT, P).transpose(2, 0, 1).reshape(P, C * FT)
        .astype(np.float32)
    )

    in_maps = []
    for i in range(NCORES):
        sl = slice(i * C_LOC, (i + 1) * C_LOC)
        in_maps.append({
            "xt": np.ascontiguousarray(xt_all[sl]),
            "wt": np.ascontiguousarray(wt_all[sl]),
            "bias": np.ascontiguousarray(
                bias_all[:, i * C_LOC * FT:(i + 1) * C_LOC * FT]
            ),
        })
    return in_maps


def _unshard_output(results):
    # per-core out: (C_LOC, N_CHUNKS, P, FT*NCH) bf16
    yt = np.stack([np.asarray(r["out"]) for r in results])
    # (NCORES, C_LOC, nb, p, ft, n) -> (C, ft, p, nb, n) == (C, F, BN)
    yt = (
        yt.reshape(C, N_CHUNKS, P, FT, NCH)
        .transpose(0, 3, 2, 1, 4)
        .reshape(C, F, B, N)
    )
    y = yt.transpose(2, 0, 3, 1).astype(np.float32)  # (B, C, N, F)
    return np.ascontiguousarray(y)


def _ensure_axon_hooks():
    """bass_utils hard-imports antenv.axon_hooks when tracing is requested;
    some images lack that module. Shim it (with the ctypes NTFF hook when
    available) only if the real module is absent."""
    try:
        import antenv.axon_hooks  # noqa: F401
        return
    except ImportError:
        pass
    import types

    import antenv

    mod = types.ModuleType("antenv.axon_hooks")
    _hook = [None]
    mod.set_axon_ntff_profile_hook = lambda h: _hook.__setitem__(0, h)
    mod.get_axon_ntff_profile_hook = lambda: _hook[0]
    sys.modules["antenv.axon_hooks"] = mod
    antenv.axon_hooks = mod
    try:
        from trn_agent_boot.trn_boot import _ntff_profile_via_ctypes

        mod.set_axon_ntff_profile_hook(
            _ntff_profile_via_ctypes("/opt/axon/libaxon_pjrt.so")
        )
    except Exception:
        pass


def run_on_device(in_maps, **kwargs):
    _ensure_axon_hooks()
    from concourse.bass_utils import run_bass_kernel_spmd

    nc = _get_compiled()
    return run_bass_kernel_spmd(nc, in_maps, core_ids=list(range(NCORES)), **kwargs)


def kernel(x, weights, bias):
    in_maps = _shard_inputs(x, weights, bias)
    res = run_on_device(in_maps)
    return _unshard_output(res.results)


# revision 17
# speedup vs baseline: 1.0093x; 1.0042x over previous
"""Channel-wise FC kernel for Trainium2 (8 NeuronCores, SPMD).

Problem: out[b,c] = x[b,c] @ weights[c].T + bias[c]
  x: (8, 32, 1024, 512) f32, weights: (32, 512, 512) f32, bias: (32, 512) f32

Sharding: channel-parallel — core i owns channels [4i, 4i+4). For each channel
the device computes YT[f, bn] = sum_g WT[g,f] * XT[g, bn] (+bias), i.e. the
output is produced f-major; the host does all layout transposes (free wrt HW
time). Device-side DRAM layouts are chosen so every DMA reads/writes long
contiguous per-partition lines:

  xt  [C_LOC, N_CHUNKS, P, GT*NCH]  xt[c,nb,p,gt*NCH+n] = x^T[c, gt*128+p, nb*NCH+n]
  wt  [C_LOC, P, GT*F]              wt[c,p,gt*F+f]      = W[c, f, gt*128+p]
  bias[P, C_LOC*FT]                 bias[p, c*FT+ft]    = bias[c, ft*128+p]
  out [C_LOC, N_CHUNKS, P, FT*NCH]  out[c,nb,p,ft*NCH+n] = Y^T[c, ft*128+p, nb*NCH+n]

Compute dtype bf16 (TensorE full rate), fp32 PSUM accumulation, bf16 stores
upcast on host.
"""

import os
import sys

for _p in ("/root/.axon_site/_ro/trn_rl_repo", "/opt/trn_rl_repo"):
    if os.path.isdir(_p) and _p not in sys.path:
        sys.path.append(_p)

import numpy as np
import ml_dtypes

B, C, N, F, G = 8, 32, 1024, 512, 512
NCORES = 8
C_LOC = C // NCORES          # 4 channels per core
BN = B * N                   # 8192 rows per channel
P = 128
GT = G // P                  # 4 contraction tiles
FT = F // P                  # 4 output-partition tiles
NCH = 2048                   # rows per x DMA chunk (16KB partition lines)
N_CHUNKS = BN // NCH         # 4
NSL = NCH // 512             # 512-row matmul slices per chunk

_BF16 = ml_dtypes.bfloat16

_compiled = None


def _build():
    import concourse.bacc as bacc
    import concourse.mybir as mybir
    import concourse.tile as tile

    BF16 = mybir.dt.bfloat16
    F32 = mybir.dt.float32

    nc = bacc.Bacc("TRN2", target_bir_lowering=False, debug=False)
    xt = nc.dram_tensor("xt", [C_LOC, N_CHUNKS, P, GT * NCH], BF16,
                        kind="ExternalInput")
    wt = nc.dram_tensor("wt", [C_LOC, P, GT * F], BF16, kind="ExternalInput")
    bias = nc.dram_tensor("bias", [P, C_LOC * FT], F32, kind="ExternalInput")
    out = nc.dram_tensor("out", [C_LOC, N_CHUNKS, P, FT * NCH], BF16,
                         kind="ExternalOutput")

    xt_ap = xt.ap()
    wt_ap = wt.ap()
    out_ap = out.ap()

    with tile.TileContext(nc) as tc:
        with (
            tc.tile_pool(name="wpool", bufs=2) as wpool,
            tc.tile_pool(name="xpool", bufs=4) as xpool,
            tc.tile_pool(name="opool", bufs=3) as opool,
            tc.tile_pool(name="bpool", bufs=1) as bpool,
            tc.tile_pool(name="psum", bufs=8, space="PSUM") as pspool,
        ):
            # --- PE warmup burst: ~10 dummy matmuls on scratch data run while
            # the first real DMAs are in flight, flipping the HAM clock gate
            # to 8/8 before real matmuls start.
            warm_sb = bpool.tile([P, 512], BF16)
            nc.vector.memset(warm_sb[:], 0.0)
            warm_ps = pspool.tile([P, 512], F32, tag="ps")
            for _ in range(14):
                nc.tensor.matmul(warm_ps[:], warm_sb[:, :P], warm_sb[:],
                                 start=True, stop=True)

            def evict(c, ft, src, dst):
                bcol = b_sb[:, c * FT + ft:c * FT + ft + 1]
                if ft % 2 == 0:
                    nc.scalar.activation(
                        dst, src, mybir.ActivationFunctionType.Identity,
                        bias=bcol,
                    )
                else:
                    nc.vector.tensor_scalar_add(dst, src, bcol)

            def mm_group(c, nb, ns, ft, w_sb, x_sb, o_sb):
                ps = pspool.tile([P, 512], F32, tag="ps",
                                 name=f"ps_{c}_{nb}_{ns}_{ft}")
                for gt in range(GT):
                    nc.tensor.matmul(
                        ps[:],
                        w_sb[:, gt * F + ft * P:gt * F + (ft + 1) * P],
                        x_sb[:, gt * NCH + ns * 512:gt * NCH + (ns + 1) * 512],
                        start=(gt == 0),
                        stop=(gt == GT - 1),
                    )
                evict(c, ft, ps[:],
                      o_sb[:, ft * NCH + ns * 512:ft * NCH + (ns + 1) * 512])

            b_sb = bpool.tile([P, C_LOC * FT], F32)
            NIDX = C_LOC * N_CHUNKS
            w_sbs, x_sbs = {}, {}

            def load_w(c):
                w_sbs[c] = wpool.tile([P, GT * F], BF16, tag="w", name=f"w_{c}")
                nc.sync.dma_start(w_sbs[c][:], wt_ap[c])

            def load_x(idx):
                c, nb = divmod(idx, N_CHUNKS)
                x_sbs[idx] = xpool.tile([P, GT * NCH], BF16, tag="x",
                                        name=f"x_{c}_{nb}")
                nc.sync.dma_start(x_sbs[idx][:], xt_ap[c, nb])

            # head: (w_gt, x_gt) pairs land in consumption order; bias last
            w_sbs[0] = wpool.tile([P, GT * F], BF16, tag="w", name="w_0")
            x_sbs[0] = xpool.tile([P, GT * NCH], BF16, tag="x", name="x_0_0")
            for gt in range(GT):
                nc.sync.dma_start(
                    w_sbs[0][:, gt * F:(gt + 1) * F],
                    wt_ap[0][:, gt * F:(gt + 1) * F],
                )
                nc.sync.dma_start(
                    x_sbs[0][:, gt * NCH:(gt + 1) * NCH],
                    xt_ap[0, 0][:, gt * NCH:(gt + 1) * NCH],
                )
            nc.sync.dma_start(b_sb[:], bias.ap())
            load_x(1)

            for idx in range(NIDX):
                c, nb = divmod(idx, N_CHUNKS)
                # issue the load for idx+2 (and any newly needed weights)
                # BEFORE this chunk's store enters the sync FIFO, so the
                # sequencer's sem-stall at the store never delays loads
                if idx + 2 < NIDX:
                    nxt_c = (idx + 2) // N_CHUNKS
                    if nxt_c not in w_sbs:
                        load_w(nxt_c)
                    load_x(idx + 2)
                w_sb, x_sb = w_sbs[c], x_sbs[idx]
                o_sb = opool.tile([P, FT * NCH], BF16, tag="o",
                                  name=f"o_{c}_{nb}")
                if idx < NIDX - 1:
                    for ns in range(NSL):
                        for ft in range(FT):
                            mm_group(c, nb, ns, ft, w_sb, x_sb, o_sb)
                    nc.sync.dma_start(out_ap[c, nb], o_sb[:])
                else:
                    # ft-outer so each ft's store fires as soon as its
                    # slices are done -> short tail
                    for ft in range(FT):
                        for ns in range(NSL):
                            mm_group(c, nb, ns, ft, w_sb, x_sb, o_sb)
                            lo = ft * NCH + ns * 512
                            nc.sync.dma_start(
                                out_ap[c, nb][:, lo:lo + 512],
                                o_sb[:, lo:lo + 512],
                            )
    nc.compile()
    return nc


def _get_compiled():
    global _compiled
    if _compiled is None:
        _compiled = _build()
    return _compiled


def _shard_inputs(x, weights, bias):
    """Host-side: slice channels per core, bf16-cast, and pre-transpose into
    the device DRAM layouts documented at the top of this file."""
    x = np.asarray(x, dtype=np.float32)
    weights = np.asarray(weights, dtype=np.float32)
    bias = np.asarray(bias, dtype=np.float32)

    # (B, C, N, G) -> (C, G, B*N) -> (C, GT, P, N_CHUNKS, NCH) -> (C, nb, p, gt, n)
    xt_all = (
        x.transpose(1, 3, 0, 2)
        .reshape(C, GT, P, N_CHUNKS, NCH)
        .transpose(0, 3, 2, 1, 4)
        .reshape(C, N_CHUNKS, P, GT * NCH)
        .astype(_BF16)
    )
    # (C, F, G) -> W^T (C, G, F) -> (C, GT, P, F) -> (C, p, gt, F)
    wt_all = (
        weights.transpose(0, 2, 1)
        .reshape(C, GT, P, F)
        .transpose(0, 2, 1, 3)
        .reshape(C, P, GT * F)
        .astype(_BF16)
    )
    # (C, F) -> (C, FT, P) -> (P, C, FT)
    bias_all = (
        bias.reshape(C, FT, P).transpose(2, 0, 1).reshape(P, C * FT)
        .astype(np.float32)
    )

    in_maps = []
    for i in range(NCORES):
        sl = slice(i * C_LOC, (i + 1) * C_LOC)
        in_maps.append({
            "xt": np.ascontiguousarray(xt_all[sl]),
            "wt": np.ascontiguousarray(wt_all[sl]),
            "bias": np.ascontiguousarray(
                bias_all[:, i * C_LOC * FT:(i + 1) * C_LOC * FT]
            ),
        })
    return in_maps


def _unshard_output(results):
    # per-core out: (C_LOC, N_CHUNKS, P, FT*NCH) bf16
    yt = np.stack([np.asarray(r["out"]) for r in results])
    # (NCORES, C_LOC, nb, p, ft, n) -> (C, ft, p, nb, n) == (C, F, BN)
    yt = (
        yt.reshape(C, N_CHUNKS, P, FT, NCH)
        .transpose(0, 3, 2, 1, 4)
        .reshape(C, F, B, N)
    )
    y = yt.transpose(2, 0, 3, 1).astype(np.float32)  # (B, C, N, F)
    return np.ascontiguousarray(y)


def _ensure_axon_hooks():
    """bass_utils hard-imports antenv.axon_hooks when tracing is requested;
    some images lack that module. Shim it (with the ctypes NTFF hook when
    available) only if the real module is absent."""
    try:
        import antenv.axon_hooks  # noqa: F401
        return
    except ImportError:
        pass
    import types

    import antenv

    mod = types.ModuleType("antenv.axon_hooks")
    _hook = [None]
    mod.set_axon_ntff_profile_hook = lambda h: _hook.__setitem__(0, h)
    mod.get_axon_ntff_profile_hook = lambda: _hook[0]
    sys.modules["antenv.axon_hooks"] = mod
    antenv.axon_hooks = mod
    try:
        from trn_agent_boot.trn_boot import _ntff_profile_via_ctypes

        mod.set_axon_ntff_profile_hook(
            _ntff_profile_via_ctypes("/opt/axon/libaxon_pjrt.so")
        )
    except Exception:
        pass


def run_on_device(in_maps, **kwargs):
    _ensure_axon_hooks()
    from concourse.bass_utils import run_bass_kernel_spmd

    nc = _get_compiled()
    return run_bass_kernel_spmd(nc, in_maps, core_ids=list(range(NCORES)), **kwargs)


def kernel(x, weights, bias):
    in_maps = _shard_inputs(x, weights, bias)
    res = run_on_device(in_maps)
    return _unshard_output(res.results)
